# revision 1
# baseline (speedup 1.0000x reference)
"""KNN classification kernel for Trainium2 (8 NeuronCores).

Problem: B=1024 queries x N=200000 gallery, D=256, top-10 neighbors,
softmax-weighted one-hot class scores over 50 classes.

Math fold: reference computes gallery = l2norm(train.T, axis=1) -- i.e. each
feature dim d is normalized by ||train[:, d]|| over the FULL gallery. That
scale folds into the query side:
    sim[b, n] = sum_d (q[b,d]/||q[b]||) * train[n,d] / ||train[:,d]||
              = q_scaled[b] . train[n]
so the device kernel is a pure matmul + top-k screen.

Device (per core, gallery sharded along N into 8 x 25000, zero-padded to
25088 = 49 x 512):
  PE: sim tile [128q, 512n] = q_scaled_bf16.T @ gallery_bf16 (2 K=128 steps),
      two tiles packed into one 2-bank PSUM slot [128, 1024]
  DVE: top-8 values per 1024-col region (InstMax) -> cand [1024, 25*8]
Host: screen top-J candidate values -> identify regions -> recompute those
  regions' sims exactly in f64 -> exact top-10 -> softmax -> class scores.
Safety: a true top-10 item's region has region-max >= item value, so the
region ranks <=10 among all regions by top value -- top-J>=16 region
screening provably covers the true top-10 (modulo bf16 noise, which is
~40 sigma below the rank-10/16 value gaps; verified empirically).
"""

import os
import numpy as np

NB_KNN = 10
T = 0.07
NUM_CLASSES = 50
EPS = 1e-12

B, N, D = 1024, 200000, 256
NCORES = 8
NPC = N // NCORES          # 25000 real cols per core
TILE = 512
NPC_PAD = 25088            # 49 * 512
NT = NPC_PAD // TILE       # 49 tiles per core
BLOCKS = [8, 8, 8, 8, 8, 8, 1]   # tiles per DMA block
NREG = 13                  # 12 grouped regions (2048 cols) + 1 single (512)
TOPJ = 16                  # regions screened per query
GROUP = 4                  # psum tiles per DVE max8 region

_CACHE = {}


def _build_bass():
    import concourse.bacc as bacc
    import concourse.tile as tile
    from concourse import mybir

    nc = bacc.Bacc("TRN2")
    bf16 = mybir.dt.bfloat16
    f32 = mybir.dt.float32

    g_d = nc.dram_tensor("g", [2, 128, NPC_PAD], bf16, kind="ExternalInput")
    q_d = nc.dram_tensor("q", [2, 128, B], bf16, kind="ExternalInput")
    cand_d = nc.dram_tensor("cand", [B, NREG * 8], f32, kind="ExternalOutput")

    with tile.TileContext(nc) as tc:
        with tc.tile_pool(name="qp", bufs=1) as qp, \
             tc.tile_pool(name="gp", bufs=2) as gp, \
             tc.tile_pool(name="cp", bufs=8) as cp, \
             tc.tile_pool(name="pp", bufs=1, space="PSUM") as pp:
            q0 = qp.tile([128, B], bf16, tag="q0")
            q1 = qp.tile([128, B], bf16, tag="q1")
            nc.sync.dma_start(out=q0[:], in_=q_d[0])
            nc.sync.dma_start(out=q1[:], in_=q_d[1])

            cands = [cp.tile([128, NREG * 8], f32, tag="cand",
                             name=f"cand{i}") for i in range(8)]

            tbase = 0
            for blk, ntile in enumerate(BLOCKS):
                cw = ntile * TILE
                c0 = tbase * TILE
                g0 = gp.tile([128, cw], bf16, tag=f"g0_{ntile}")
                g1 = gp.tile([128, cw], bf16, tag=f"g1_{ntile}")
                nc.sync.dma_start(out=g0[:], in_=g_d[0][:, c0:c0 + cw])
                nc.sync.dma_start(out=g1[:], in_=g_d[1][:, c0:c0 + cw])
                for bc in range(8):
                    lhs0 = q0[:, bc * 128:(bc + 1) * 128]
                    lhs1 = q1[:, bc * 128:(bc + 1) * 128]
                    for p in range(0, ntile, GROUP):
                        grp = min(GROUP, ntile - p)
                        ps = pp.tile([128, TILE * GROUP], f32,
                                     tag="ps4", bufs=2)
                        for s in range(grp):
                            rsl = slice((p + s) * TILE, (p + s + 1) * TILE)
                            osl = slice(s * TILE, (s + 1) * TILE)
                            nc.tensor.matmul(ps[:, osl], lhs0, g0[:, rsl],
                                             start=True, stop=False)
                            nc.tensor.matmul(ps[:, osl], lhs1, g1[:, rsl],
                                             start=False, stop=True)
                        reg = (tbase + p) // GROUP
                        nc.vector.max(cands[bc][:, reg * 8:(reg + 1) * 8],
                                      ps[:, :TILE * grp])
                tbase += ntile

            for bc in range(8):
                nc.sync.dma_start(
                    out=cand_d[bc * 128:(bc + 1) * 128, :], in_=cands[bc][:])
    if not nc.is_finalized():
        nc.finalize()
    return nc


def _run_device(g_shards, q_packed):
    from concourse.bass_utils import run_bass_kernel_spmd
    if "nc" not in _CACHE:
        _CACHE["nc"] = _build_bass()
    nc = _CACHE["nc"]
    in_maps = [{"g": g_shards[c], "q": q_packed} for c in range(NCORES)]
    res = run_bass_kernel_spmd(nc, in_maps, list(range(NCORES)))
    return np.concatenate(
        [res.results[c]["cand"] for c in range(NCORES)], axis=1)


def _run_emulated(g_shards, q_packed):
    qf = q_packed.astype(np.float32).reshape(256, B)
    out = []
    for c in range(NCORES):
        gf = g_shards[c].astype(np.float32).reshape(256, NPC_PAD)
        sim = qf.T @ gf                                   # [B, NPC_PAD]
        res = np.empty((B, NREG * 8), np.float32)
        for r in range(NREG):
            a = r * 2048
            b = min(a + 2048, NPC_PAD)
            blkv = sim[:, a:b]
            top8 = -np.sort(-blkv, axis=1)[:, :8]
            res[:, r * 8:(r + 1) * 8] = top8
        out.append(res)
    return np.concatenate(out, axis=1)


def kernel(test_features, train_features, train_labels):
    test_features = np.asarray(test_features, dtype=np.float32)
    train_features = np.asarray(train_features, dtype=np.float32)
    train_labels = np.asarray(train_labels)

    import ml_dtypes
    bf16 = ml_dtypes.bfloat16

    # ---- host pre: fold normalizations into the query side ----
    tf64 = train_features.astype(np.float64)
    norm_d = np.maximum(np.sqrt(np.sum(tf64 * tf64, axis=0)), EPS)
    q64 = test_features.astype(np.float64)
    qn = np.sqrt(np.sum(q64 * q64, axis=1, keepdims=True))
    q_scaled = q64 / np.maximum(qn, EPS) / norm_d          # [B, D] f64

    q_packed = np.ascontiguousarray(
        q_scaled.T.astype(bf16).reshape(2, 128, B))
    gt = train_features.T.astype(bf16)                     # [D, N]
    g_shards = []
    for c in range(NCORES):
        sl = np.zeros((256, NPC_PAD), dtype=bf16)
        sl[:, :NPC] = gt[:, c * NPC:(c + 1) * NPC]
        g_shards.append(np.ascontiguousarray(sl.reshape(2, 128, NPC_PAD)))

    # ---- device: bf16 matmul + per-region top-8 screen ----
    if os.environ.get("KNN_EMULATE"):
        cand = _run_emulated(g_shards, q_packed)
    else:
        cand = _run_device(g_shards, q_packed)
    cand = cand.astype(np.float32)                         # [B, 8*NREG*8]

    # ---- host post: screen -> exact f64 rerank -> softmax scores ----
    topj = np.argpartition(-cand, TOPJ - 1, axis=1)[:, :TOPJ]
    reg_id = topj // 8                                     # 0..199 global

    reg_queries = {}
    for b in range(B):
        for r in set(reg_id[b].tolist()):
            reg_queries.setdefault(r, []).append(b)

    per_q_vals = [[] for _ in range(B)]
    per_q_cols = [[] for _ in range(B)]
    for r, qs in reg_queries.items():
        core, rc = divmod(r, NREG)
        c0 = core * NPC + rc * 2048
        c1 = core * NPC + min(rc * 2048 + 2048, NPC)
        block = tf64[c0:c1]                                # [w, D] view
        sims = q_scaled[qs] @ block.T                      # [nq, w] f64
        cols = np.arange(c0, c1)
        for i, b in enumerate(qs):
            per_q_vals[b].append(sims[i])
            per_q_cols[b].append(cols)

    labels = train_labels.astype(np.int64)
    scores = np.zeros((B, NUM_CLASSES), dtype=np.float64)
    for b in range(B):
        v = np.concatenate(per_q_vals[b])
        cidx = np.concatenate(per_q_cols[b])
        sel = np.argpartition(-v, NB_KNN - 1)[:NB_KNN]
        order = np.lexsort((cidx[sel], -v[sel]))
        sel = sel[order]
        topv = v[sel]
        w = np.exp(topv / T - np.max(topv) / T)
        w /= w.sum()
        np.add.at(scores[b], labels[cidx[sel]], w)
    return scores.astype(np.float32)


if __name__ == "__main__":
    rng = np.random.default_rng(0)
    tf = rng.standard_normal((B, D), dtype=np.float32)
    trf = rng.standard_normal((N, D), dtype=np.float32)
    trl = rng.integers(0, NUM_CLASSES, N).astype(np.int64)
    os.environ["KNN_EMULATE"] = "1"
    out = kernel(tf, trf, trl)
    print(out.shape, out.dtype, out.sum())



# revision 4
# speedup vs baseline: 1.4939x; 1.4939x over previous
"""KNN classification kernel for Trainium2 (8 NeuronCores).

Problem: B=1024 queries x N=200000 gallery, D=256, top-10 neighbors,
softmax-weighted one-hot class scores over 50 classes.

Math fold: reference computes gallery = l2norm(train.T, axis=1) -- i.e. each
feature dim d is normalized by ||train[:, d]|| over the FULL gallery. That
scale folds into the query side:
    sim[b, n] = sum_d (q[b,d]/||q[b]||) * train[n,d] / ||train[:,d]||
              = q_scaled[b] . train[n]
so the device kernel is a pure matmul + top-k screen.

Device (per core, gallery sharded along N into 8 x 25000, zero-padded to
25088 = 49 x 512), all in fp8-e4m3 with DoubleRow perf mode (K=256 folded
into one PE instruction at 0.5 cycles/row):
  PE:  sim tile [128q, 512n] per matmul, 4 tiles packed into a [128, 2048]
       PSUM region (2 regions in flight = all 8 banks).
  The per-region screen is split across two engines so neither is the
  bottleneck (the sims can only leave PSUM through DVE or Act):
    DVE: top-8 values per even 2048-col span (InstMax)  -> cand [1024, 6*8]
    Act: hinge mass acc = sum(relu(sim - t)) per odd span (+ the 512 tail)
         -> acc [1024, 7].  t + acc >= span max, a sound upper bound.
Host: per query take tau = 10th-largest DVE witness value; exactly rerank
  (f64) every span whose upper bound >= tau - mu (mu covers fp8 noise,
  measured sigma 0.032*sim_sigma; mu = 450 device units ~ 12 noise sigmas);
  exact top-10 -> softmax -> class scores.
Safety: every true top-10 item's span has UB >= its device sim; tau is a
  real device sim lower bound, so the margin only needs to cover fp8 noise
  on both sides.
"""

import os
import numpy as np

NB_KNN = 10
T = 0.07
NUM_CLASSES = 50
EPS = 1e-12

B, N, D = 1024, 200000, 256
NCORES = 8
NPC = N // NCORES          # 25000 real cols per core
TILE = 512
NPC_PAD = 25088            # 49 * 512
NT = NPC_PAD // TILE       # 49 tiles per core
BLOCKS = [8, 8, 8, 8, 8, 8, 1]   # tiles per DMA block
NSPAN = 13                 # 12 x 2048-col spans + 1 x 512-col tail per core
SPAN_W = 2048
DVE_SPANS = (0, 2, 4, 6, 8, 10)        # top-8 screened spans
ACT_SPANS = (1, 3, 5, 7, 9, 11, 12)    # hinge-accum screened spans
NDVE = len(DVE_SPANS)
NACT = len(ACT_SPANS)

SQ = 64.0                  # query fp8 pre-scale
SG = 16.0                  # gallery fp8 pre-scale
T_DEV = 3946.0             # hinge threshold, ~3.3 sim-sigma in device units
MU = 450.0                 # fp8-noise screen margin, device units

_CACHE = {}


def _build_bass():
    import concourse.bacc as bacc
    import concourse.tile as tile
    from concourse import mybir

    nc = bacc.Bacc("TRN2")
    f8 = mybir.dt.float8e4
    bf16 = mybir.dt.bfloat16
    f32 = mybir.dt.float32
    DR = mybir.MatmulPerfMode.DoubleRow

    g_d = nc.dram_tensor("g", [2, 128, NPC_PAD], f8, kind="ExternalInput")
    q_d = nc.dram_tensor("q", [2, 128, B], f8, kind="ExternalInput")
    cand_d = nc.dram_tensor("cand", [B, NDVE * 8], f32, kind="ExternalOutput")
    acc_d = nc.dram_tensor("acc", [B, NACT], f32, kind="ExternalOutput")

    with tile.TileContext(nc) as tc:
        with tc.tile_pool(name="qp", bufs=1) as qp, \
             tc.tile_pool(name="gp", bufs=2) as gp, \
             tc.tile_pool(name="cp", bufs=8) as cp, \
             tc.tile_pool(name="sp", bufs=2) as sp, \
             tc.tile_pool(name="pp", bufs=1, space="PSUM") as pp:
            q8 = qp.tile([128, 2, B], f8, tag="q8")
            nc.sync.dma_start(out=q8[:, 0, :], in_=q_d[0])
            nc.sync.dma_start(out=q8[:, 1, :], in_=q_d[1])
            bias_t = qp.tile([128, 1], f32, tag="bias")
            nc.vector.memset(bias_t[:], -T_DEV)

            cands = [cp.tile([128, NDVE * 8], f32, tag="cand",
                             name=f"cand{i}") for i in range(8)]
            accs = [cp.tile([128, NACT], f32, tag="acc",
                            name=f"acc{i}") for i in range(8)]

            tbase = 0
            for blk, ntile in enumerate(BLOCKS):
                cw = ntile * TILE
                c0 = tbase * TILE
                g8 = gp.tile([128, 2, cw], f8, tag=f"g8_{ntile}")
                nc.sync.dma_start(out=g8[:, 0, :], in_=g_d[0][:, c0:c0 + cw])
                nc.sync.dma_start(out=g8[:, 1, :], in_=g_d[1][:, c0:c0 + cw])
                nspan_blk = (ntile * TILE + SPAN_W - 1) // SPAN_W
                for bc in range(8):
                    lhs = q8[:, :, bc * 128:(bc + 1) * 128]
                    for sp_i in range(nspan_blk):
                        span = (tbase * TILE) // SPAN_W + sp_i
                        off = sp_i * SPAN_W
                        w = min(SPAN_W, cw - off)
                        ps = pp.tile([128, SPAN_W], f32, tag="ps", bufs=2)
                        for s in range(w // TILE):
                            osl = slice(s * TILE, (s + 1) * TILE)
                            gsl = slice(off + s * TILE, off + (s + 1) * TILE)
                            nc.tensor.matmul(ps[:, osl], lhs, g8[:, :, gsl],
                                             start=True, stop=True,
                                             perf_mode=DR)
                        if span in DVE_SPANS:
                            k = DVE_SPANS.index(span)
                            nc.vector.max(cands[bc][:, k * 8:(k + 1) * 8],
                                          ps[:, :w])
                        else:
                            j = ACT_SPANS.index(span)
                            scr = sp.tile([128, SPAN_W], bf16, tag="scr")
                            nc.scalar.activation(
                                out=scr[:, :w], in_=ps[:, :w],
                                func=mybir.ActivationFunctionType.Relu,
                                bias=bias_t[:], scale=1.0,
                                accum_out=accs[bc][:, j:j + 1])
                tbase += ntile

            for bc in range(8):
                nc.sync.dma_start(
                    out=cand_d[bc * 128:(bc + 1) * 128, :], in_=cands[bc][:])
                nc.sync.dma_start(
                    out=acc_d[bc * 128:(bc + 1) * 128, :], in_=accs[bc][:])
    if not nc.is_finalized():
        nc.finalize()
    return nc


def _run_device(g_shards, q_packed):
    from concourse.bass_utils import run_bass_kernel_spmd
    if "nc" not in _CACHE:
        _CACHE["nc"] = _build_bass()
    nc = _CACHE["nc"]
    in_maps = [{"g": g_shards[c], "q": q_packed} for c in range(NCORES)]
    res = run_bass_kernel_spmd(nc, in_maps, list(range(NCORES)))
    cand = np.concatenate(
        [res.results[c]["cand"] for c in range(NCORES)], axis=1)
    acc = np.concatenate(
        [res.results[c]["acc"] for c in range(NCORES)], axis=1)
    return cand, acc


def _run_emulated(g_shards, q_packed):
    qf = q_packed.astype(np.float32).reshape(256, B)
    cands, accs = [], []
    for c in range(NCORES):
        gf = g_shards[c].astype(np.float32).reshape(256, NPC_PAD)
        sim = qf.T @ gf                                   # [B, NPC_PAD]
        cd = np.empty((B, NDVE * 8), np.float32)
        ac = np.empty((B, NACT), np.float32)
        for k, s in enumerate(DVE_SPANS):
            blkv = sim[:, s * SPAN_W:(s + 1) * SPAN_W]
            cd[:, k * 8:(k + 1) * 8] = -np.sort(-blkv, axis=1)[:, :8]
        for j, s in enumerate(ACT_SPANS):
            blkv = sim[:, s * SPAN_W:min((s + 1) * SPAN_W, NPC_PAD)]
            h = np.maximum(blkv.astype(np.float32) - np.float32(T_DEV), 0)
            ac[:, j] = np.sum(h.astype(np.float32), axis=1)
        cands.append(cd)
        accs.append(ac)
    return np.concatenate(cands, axis=1), np.concatenate(accs, axis=1)


def kernel(test_features, train_features, train_labels):
    import ml_dtypes
    f8 = ml_dtypes.float8_e4m3

    test_features = np.asarray(test_features, dtype=np.float32)
    train_features = np.asarray(train_features, dtype=np.float32)
    train_labels = np.asarray(train_labels)

    # ---- host pre: fold normalizations into the query side ----
    tf64 = train_features.astype(np.float64)
    norm_d = np.maximum(np.sqrt(np.sum(tf64 * tf64, axis=0)), EPS)
    q64 = test_features.astype(np.float64)
    qn = np.sqrt(np.sum(q64 * q64, axis=1, keepdims=True))
    q_scaled = q64 / np.maximum(qn, EPS) / norm_d          # [B, D] f64
    # unit-normalized device queries: device sims are the same ranking per
    # query as q_scaled . t_n, at a benign per-query positive scale
    row = np.sqrt(np.sum(q_scaled * q_scaled, axis=1, keepdims=True))
    q_unit = q_scaled / np.maximum(row, EPS)

    q_packed = np.ascontiguousarray(
        (q_unit.T * SQ).astype(f8).reshape(2, 128, B))
    gt8 = (train_features.T * SG).astype(f8)               # [D, N]
    g_shards = []
    for c in range(NCORES):
        sl = np.zeros((256, NPC_PAD), dtype=f8)
        sl[:, :NPC] = gt8[:, c * NPC:(c + 1) * NPC]
        g_shards.append(np.ascontiguousarray(sl.reshape(2, 128, NPC_PAD)))

    # ---- device: fp8 matmul + split DVE/Act screen ----
    if os.environ.get("KNN_EMULATE"):
        cand, acc = _run_emulated(g_shards, q_packed)
    else:
        cand, acc = _run_device(g_shards, q_packed)
    cand = cand.astype(np.float64)            # [B, 8*NDVE*8] DVE witnesses
    acc = acc.astype(np.float64)              # [B, 8*NACT] hinge masses

    # ---- host post: screen -> exact f64 rerank -> softmax scores ----
    # per-query witness threshold: 10th-largest real device sim seen by DVE
    tau = -np.sort(-cand, axis=1)[:, NB_KNN - 1]           # [B]
    thresh = tau - MU

    # upper bounds per span: DVE spans -> top-1 value; Act spans -> t + acc
    dve_top1 = cand.reshape(B, NCORES, NDVE, 8)[:, :, :, 0]
    act_ub = T_DEV + acc.reshape(B, NCORES, NACT)

    # global span table: (core, span) -> column range
    span_cols = []
    for c in range(NCORES):
        for s in range(NSPAN):
            a = c * NPC + s * SPAN_W
            b_ = c * NPC + min(s * SPAN_W + SPAN_W, NPC)
            span_cols.append((a, b_))

    sel = np.zeros((B, NCORES * NSPAN), dtype=bool)
    for k, s in enumerate(DVE_SPANS):
        sel[:, np.arange(NCORES) * NSPAN + s] = \
            dve_top1[:, :, k] >= thresh[:, None]
    for j, s in enumerate(ACT_SPANS):
        sel[:, np.arange(NCORES) * NSPAN + s] = \
            act_ub[:, :, j] >= thresh[:, None]

    reg_queries = {}
    for b in range(B):
        for r in np.nonzero(sel[b])[0]:
            reg_queries.setdefault(int(r), []).append(b)

    per_q_vals = [[] for _ in range(B)]
    per_q_cols = [[] for _ in range(B)]
    for r, qs in reg_queries.items():
        c0, c1 = span_cols[r]
        if c0 >= c1:
            continue
        block = tf64[c0:c1]                                # [w, D] view
        sims = q_scaled[qs] @ block.T                      # [nq, w] f64
        cols = np.arange(c0, c1)
        for i, b in enumerate(qs):
            per_q_vals[b].append(sims[i])
            per_q_cols[b].append(cols)

    labels = train_labels.astype(np.int64)
    scores = np.zeros((B, NUM_CLASSES), dtype=np.float64)
    for b in range(B):
        v = np.concatenate(per_q_vals[b])
        cidx = np.concatenate(per_q_cols[b])
        sel_i = np.argpartition(-v, NB_KNN - 1)[:NB_KNN]
        order = np.lexsort((cidx[sel_i], -v[sel_i]))
        sel_i = sel_i[order]
        topv = v[sel_i]
        w = np.exp(topv / T - np.max(topv) / T)
        w /= w.sum()
        np.add.at(scores[b], labels[cidx[sel_i]], w)
    return scores.astype(np.float32)


if __name__ == "__main__":
    rng = np.random.default_rng(0)
    tf = rng.standard_normal((B, D), dtype=np.float32)
    trf = rng.standard_normal((N, D), dtype=np.float32)
    trl = rng.integers(0, NUM_CLASSES, N).astype(np.int64)
    os.environ["KNN_EMULATE"] = "1"
    out = kernel(tf, trf, trl)
    print(out.shape, out.dtype, out.sum())


# revision 10
# speedup vs baseline: 1.8430x; 1.2337x over previous
"""KNN classification kernel for Trainium2 (8 NeuronCores).

Problem: B=1024 queries x N=200000 gallery, D=256, top-10 neighbors,
softmax-weighted one-hot class scores over 50 classes.

Math fold: reference computes gallery = l2norm(train.T, axis=1) -- i.e. each
feature dim d is normalized by ||train[:, d]|| over the FULL gallery. That
scale folds into the query side, so the device computes a pure matmul plus a
top-k screen; the host reranks screened spans exactly in f64.

Device (per core, gallery sharded along N into 8 x 25000, zero-padded to
25088 = 49 x 512), all in fp8-e4m3 with DoubleRow perf mode (K=256 folded
into one PE instruction at 0.5 cycles/row).  Sims live in PSUM and can only
leave through DVE or Act, so the screen is split so neither engine is the
bottleneck; per 128-query batch, the 24.5 1024-col chunks go to two lanes
with independent double-buffered PSUM tags (so the chains don't serialize):
  DVE lane (11 chunks):    top-8 values per chunk (InstMax)
  Act lane  (13.5 chunks): bf16 copy to SBUF, then DMA to DRAM; the host
                           screens those sims directly (bf16 error ~8 device
                           units << margin)
Host: tau_q = 10th-largest device sim among all witnesses (DVE top-8s and
  dumped bf16 sims); exactly rerank (f64) every span whose max-witness >=
  tau - mu (mu = 450 device units ~ 12 sigmas of the measured fp8 matmul
  noise); exact top-10 -> softmax -> class scores, identical math to the
  reference.
"""

import os
import numpy as np

NB_KNN = 10
T = 0.07
NUM_CLASSES = 50
EPS = 1e-12

B, N, D = 1024, 200000, 256
NCORES = 8
NPC = N // NCORES          # 25000 real cols per core
TILE = 512
NPC_PAD = 25088            # 49 * 512
CHUNK = 1024               # consumer chunk width
NCH = 24                   # full 1024-chunks per core (+ one 512 tail)
BLOCKS = [8, 8, 8, 8, 8, 8, 1]   # 512-tiles per gallery DMA block

SQ = 64.0                  # query fp8 pre-scale
SG = 16.0                  # gallery fp8 pre-scale
MU = 450.0                 # fp8-noise screen margin, device units

# chunk lanes per 4-chunk gallery block: A,A,D,D x5 blocks, then A,A,A,D,
# then the 512 tail -> A.  A pairs share one dump DMA.
_LANES = (["A", "A", "D", "D"] * 5) + ["A", "A", "A", "D"]
D_CHUNKS = [i for i, l in enumerate(_LANES) if l == "D"]   # 11 chunks
A_CHUNKS = [i for i, l in enumerate(_LANES) if l == "A"]   # 13 chunks
ND = len(D_CHUNKS)
NA = len(A_CHUNKS)
ARAW_W = NA * CHUNK + TILE   # 13824 bf16 sims dumped per (query, core)

_CACHE = {}


def _build_bass():
    import concourse.bacc as bacc
    import concourse.tile as tile
    from concourse import mybir

    nc = bacc.Bacc("TRN2")
    f8 = mybir.dt.float8e4
    bf16 = mybir.dt.bfloat16
    f32 = mybir.dt.float32
    DR = mybir.MatmulPerfMode.DoubleRow
    Copy = mybir.ActivationFunctionType.Copy

    g_d = nc.dram_tensor("g", [2, 128, NPC_PAD], f8, kind="ExternalInput")
    q_d = nc.dram_tensor("q", [2, 128, B], f8, kind="ExternalInput")
    cand_d = nc.dram_tensor("cand", [B, ND * 8], f32, kind="ExternalOutput")
    araw_d = nc.dram_tensor("araw", [B, ARAW_W], bf16, kind="ExternalOutput")

    with tile.TileContext(nc) as tc:
        with tc.tile_pool(name="qp", bufs=1) as qp, \
             tc.tile_pool(name="gp", bufs=2) as gp, \
             tc.tile_pool(name="cp", bufs=8) as cp, \
             tc.tile_pool(name="sp", bufs=4) as sp, \
             tc.tile_pool(name="pp", bufs=1, space="PSUM") as pp:
            q8 = qp.tile([128, 2, B], f8, tag="q8")
            nc.sync.dma_start(out=q8[:, 0, :], in_=q_d[0])
            nc.sync.dma_start(out=q8[:, 1, :], in_=q_d[1])

            cands = [cp.tile([128, ND * 8], f32, tag="cand",
                             name=f"cand{i}") for i in range(8)]

            def consume(bc, ci, w, lhs, g8, goff, pend):
                """Emit matmuls for chunk ci (width w) and its consumer."""
                lane = "A" if (ci >= NCH or _LANES[ci] == "A") else "D"
                ps = pp.tile([128, CHUNK], f32, tag=f"ps{lane}", bufs=2,
                             name=f"ps{lane}_{bc}_{ci}")
                for s in range(w // TILE):
                    nc.tensor.matmul(
                        ps[:, s * TILE:(s + 1) * TILE], lhs,
                        g8[:, :, goff + s * TILE:goff + (s + 1) * TILE],
                        start=True, stop=True, perf_mode=DR)
                if lane == "D":
                    k = D_CHUNKS.index(ci)
                    nc.vector.max(cands[bc][:, k * 8:(k + 1) * 8],
                                  ps[:, :w])
                    return pend
                # Act lane: bf16 copy, buffer into a paired scratch, dump
                # one DMA per completed pair (or at a lone/tail chunk).
                ai = NA if ci >= NCH else A_CHUNKS.index(ci)
                half = ai % 2 if ai < NA else 0
                if pend is None:
                    pend = (sp.tile([128, 2 * CHUNK], bf16, tag="scr",
                                    name=f"scr_{bc}_{ci}"), [])
                scr, slots = pend
                nc.scalar.activation(out=scr[:, half * CHUNK:half * CHUNK + w],
                                     in_=ps[:, :w], func=Copy)
                slots.append((ai, w))
                # flush on a full pair, on the lone A (ai==12), or tail
                if half == 1 or ai >= NA - 1:
                    a0, w0 = slots[0]
                    wtot = sum(wi for _, wi in slots)
                    nc.sync.dma_start(
                        out=araw_d[bc * 128:(bc + 1) * 128,
                                   a0 * CHUNK:a0 * CHUNK + wtot],
                        in_=scr[:, :wtot])
                    return None
                return pend

            tbase = 0
            for blk, ntile in enumerate(BLOCKS):
                cw = ntile * TILE
                c0 = tbase * TILE
                g8 = gp.tile([128, 2, cw], f8, tag=f"g8_{ntile}")
                nc.sync.dma_start(out=g8[:, 0, :], in_=g_d[0][:, c0:c0 + cw])
                nc.sync.dma_start(out=g8[:, 1, :], in_=g_d[1][:, c0:c0 + cw])
                for bc in range(8):
                    lhs = q8[:, :, bc * 128:(bc + 1) * 128]
                    pend = None
                    if ntile == 1:                  # 512 tail -> Act lane
                        pend = consume(bc, NCH, TILE, lhs, g8, 0, pend)
                    else:
                        for j in range(4):          # four 1024-chunks
                            ci = (tbase * TILE) // CHUNK + j
                            pend = consume(bc, ci, CHUNK, lhs, g8,
                                           j * CHUNK, pend)
                    assert pend is None
                tbase += ntile

            for bc in range(8):
                nc.sync.dma_start(
                    out=cand_d[bc * 128:(bc + 1) * 128, :], in_=cands[bc][:])
    if not nc.is_finalized():
        nc.finalize()
    return nc


def _run_device(g_shards, q_packed):
    from concourse.bass_utils import run_bass_kernel_spmd
    if "nc" not in _CACHE:
        _CACHE["nc"] = _build_bass()
    nc = _CACHE["nc"]
    in_maps = [{"g": g_shards[c], "q": q_packed} for c in range(NCORES)]
    res = run_bass_kernel_spmd(nc, in_maps, list(range(NCORES)))
    cand = np.concatenate(
        [res.results[c]["cand"] for c in range(NCORES)], axis=1)
    araw = np.stack([res.results[c]["araw"] for c in range(NCORES)], axis=1)
    return cand, araw                     # araw: [B, NCORES, ARAW_W] bf16


def _run_emulated(g_shards, q_packed):
    import ml_dtypes
    qf = q_packed.astype(np.float32).reshape(256, B)
    cands, araws = [], []
    for c in range(NCORES):
        gf = g_shards[c].astype(np.float32).reshape(256, NPC_PAD)
        sim = qf.T @ gf                                   # [B, NPC_PAD]
        cd = np.empty((B, ND * 8), np.float32)
        for k, ci in enumerate(D_CHUNKS):
            blkv = sim[:, ci * CHUNK:(ci + 1) * CHUNK]
            cd[:, k * 8:(k + 1) * 8] = -np.sort(-blkv, axis=1)[:, :8]
        ar = np.concatenate(
            [sim[:, ci * CHUNK:(ci + 1) * CHUNK] for ci in A_CHUNKS] +
            [sim[:, NCH * CHUNK:NCH * CHUNK + TILE]], axis=1)
        cands.append(cd)
        araws.append(ar.astype(ml_dtypes.bfloat16))
    return np.concatenate(cands, axis=1), np.stack(araws, axis=1)


def kernel(test_features, train_features, train_labels):
    import ml_dtypes
    f8 = ml_dtypes.float8_e4m3

    test_features = np.asarray(test_features, dtype=np.float32)
    train_features = np.asarray(train_features, dtype=np.float32)
    train_labels = np.asarray(train_labels)

    # ---- host pre: fold normalizations into the query side ----
    tf64 = train_features.astype(np.float64)
    norm_d = np.maximum(np.sqrt(np.sum(tf64 * tf64, axis=0)), EPS)
    q64 = test_features.astype(np.float64)
    qn = np.sqrt(np.sum(q64 * q64, axis=1, keepdims=True))
    q_scaled = q64 / np.maximum(qn, EPS) / norm_d          # [B, D] f64
    # unit-normalized device queries: same per-query ranking as q_scaled
    row = np.sqrt(np.sum(q_scaled * q_scaled, axis=1, keepdims=True))
    q_unit = q_scaled / np.maximum(row, EPS)

    q_packed = np.ascontiguousarray(
        (q_unit.T * SQ).astype(f8).reshape(2, 128, B))
    gt8 = (train_features.T * SG).astype(f8)               # [D, N]
    g_shards = []
    for c in range(NCORES):
        sl = np.zeros((256, NPC_PAD), dtype=f8)
        sl[:, :NPC] = gt8[:, c * NPC:(c + 1) * NPC]
        g_shards.append(np.ascontiguousarray(sl.reshape(2, 128, NPC_PAD)))

    # ---- device: fp8 matmul + two-lane screen ----
    if os.environ.get("KNN_EMULATE"):
        cand, araw = _run_emulated(g_shards, q_packed)
    else:
        cand, araw = _run_device(g_shards, q_packed)
    cand = cand.astype(np.float64)            # [B, NCORES*ND*8]
    araw = araw.astype(np.float32)            # [B, NCORES, ARAW_W]

    # ---- host post: screen -> exact f64 rerank -> softmax scores ----
    # per-query witness threshold: 10th-largest device sim seen
    # ARAW_W = 13.5*1024 = 27*512: screen dumped sims at 512-col granularity
    a_chmax = araw.reshape(B, NCORES, 27, 512).max(axis=3)     # [B,C,27]
    dve_top1 = cand.reshape(B, NCORES, ND, 8)[:, :, :, 0]      # [B,C,ND]
    wit = np.concatenate(
        [cand, a_chmax.reshape(B, -1)], axis=1)
    tau = -np.partition(-wit, NB_KNN - 1, axis=1)[:, NB_KNN - 1]
    thresh = tau - MU                                          # [B]

    # span table: DVE chunks (1024 cols) + Act half-chunks (512 cols)
    spans = []          # (col0, col1) global
    sel_cols = []       # [B] bool per span
    for c in range(NCORES):
        base = c * NPC
        for k, ci in enumerate(D_CHUNKS):
            spans.append((base + ci * CHUNK,
                          base + min((ci + 1) * CHUNK, NPC)))
            sel_cols.append(dve_top1[:, c, k] >= thresh)
        for h in range(27):
            if h < 26:
                ci = A_CHUNKS[h // 2]
                c0 = base + ci * CHUNK + (h % 2) * 512
            else:
                c0 = base + NCH * CHUNK
            spans.append((c0, min(c0 + 512, base + NPC)))
            sel_cols.append(a_chmax[:, c, h] >= thresh)
    sel = np.stack(sel_cols, axis=1)                       # [B, nspans]

    reg_queries = {}
    for b in range(B):
        for r in np.nonzero(sel[b])[0]:
            reg_queries.setdefault(int(r), []).append(b)

    per_q_vals = [[] for _ in range(B)]
    per_q_cols = [[] for _ in range(B)]
    for r, qs in reg_queries.items():
        c0, c1 = spans[r]
        if c0 >= c1:
            continue
        block = tf64[c0:c1]                                # [w, D] view
        sims = q_scaled[qs] @ block.T                      # [nq, w] f64
        cols = np.arange(c0, c1)
        for i, b in enumerate(qs):
            per_q_vals[b].append(sims[i])
            per_q_cols[b].append(cols)

    labels = train_labels.astype(np.int64)
    scores = np.zeros((B, NUM_CLASSES), dtype=np.float64)
    for b in range(B):
        v = np.concatenate(per_q_vals[b])
        cidx = np.concatenate(per_q_cols[b])
        sel_i = np.argpartition(-v, NB_KNN - 1)[:NB_KNN]
        order = np.lexsort((cidx[sel_i], -v[sel_i]))
        sel_i = sel_i[order]
        topv = v[sel_i]
        w = np.exp(topv / T - np.max(topv) / T)
        w /= w.sum()
        np.add.at(scores[b], labels[cidx[sel_i]], w)
    return scores.astype(np.float32)


if __name__ == "__main__":
    rng = np.random.default_rng(0)
    tf = rng.standard_normal((B, D), dtype=np.float32)
    trf = rng.standard_normal((N, D), dtype=np.float32)
    trl = rng.integers(0, NUM_CLASSES, N).astype(np.int64)
    os.environ["KNN_EMULATE"] = "1"
    out = kernel(tf, trf, trl)
    print(out.shape, out.dtype, out.sum())


# revision 12
# speedup vs baseline: 1.8948x; 1.0281x over previous
"""KNN classification kernel for Trainium2 (8 NeuronCores).

Problem: B=1024 queries x N=200000 gallery, D=256, top-10 neighbors,
softmax-weighted one-hot class scores over 50 classes.

Math fold: reference computes gallery = l2norm(train.T, axis=1) -- i.e. each
feature dim d is normalized by ||train[:, d]|| over the FULL gallery. That
scale folds into the query side, so the device computes a pure matmul plus a
top-k screen; the host reranks screened spans exactly in f64.

Device (per core, gallery sharded along N into 8 x 25000, zero-padded to
25088 = 49 x 512), all in fp8-e4m3 with DoubleRow perf mode (K=256 folded
into one PE instruction at 0.5 cycles/row).  Sims live in PSUM and can only
leave through DVE or Act, so the screen is split so neither engine is the
bottleneck; per 128-query batch, the 24.5 1024-col chunks go to two lanes
with independent double-buffered PSUM tags (so the chains don't serialize):
  DVE lane (11 chunks):    top-8 values per chunk (InstMax)
  Act lane  (13.5 chunks): bf16 copy to SBUF, then DMA to DRAM; the host
                           screens those sims directly (bf16 error ~8 device
                           units << margin)
Host: tau_q = 10th-largest device sim among all witnesses (DVE top-8s and
  dumped bf16 sims); exactly rerank (f64) every span whose max-witness >=
  tau - mu (mu = 450 device units ~ 12 sigmas of the measured fp8 matmul
  noise); exact top-10 -> softmax -> class scores, identical math to the
  reference.
"""

import os
import numpy as np

NB_KNN = 10
T = 0.07
NUM_CLASSES = 50
EPS = 1e-12

B, N, D = 1024, 200000, 256
NCORES = 8
NPC = N // NCORES          # 25000 real cols per core
TILE = 512
NPC_PAD = 25088            # 49 * 512
CHUNK = 1024               # consumer chunk width
NCH = 24                   # full 1024-chunks per core (+ one 512 tail)
BLOCKS = [8, 8, 8, 8, 8, 8, 1]   # 512-tiles per gallery DMA block

SQ = 64.0                  # query fp8 pre-scale
SG = 16.0                  # gallery fp8 pre-scale
MU = 450.0                 # fp8-noise screen margin, device units

# chunk lanes per 4-chunk gallery block: A,A,D,D x5 blocks, then A,A,A,D,
# then the 512 tail -> D.  A pairs share one dump DMA.
_LANES = (["A", "A", "D", "D"] * 5) + ["A", "A", "A", "D"]
D_CHUNKS = [i for i, l in enumerate(_LANES) if l == "D"]   # 11 chunks
A_CHUNKS = [i for i, l in enumerate(_LANES) if l == "A"]   # 13 chunks
ND = len(D_CHUNKS) + 1       # + the 512 tail chunk
NA = len(A_CHUNKS)
ARAW_W = NA * CHUNK          # 13312 bf16 sims dumped per (query, core)

_CACHE = {}


def _build_bass():
    import concourse.bacc as bacc
    import concourse.tile as tile
    from concourse import mybir

    nc = bacc.Bacc("TRN2")
    f8 = mybir.dt.float8e4
    bf16 = mybir.dt.bfloat16
    f32 = mybir.dt.float32
    DR = mybir.MatmulPerfMode.DoubleRow
    Copy = mybir.ActivationFunctionType.Copy

    g_d = nc.dram_tensor("g", [2, 128, NPC_PAD], f8, kind="ExternalInput")
    q_d = nc.dram_tensor("q", [2, 128, B], f8, kind="ExternalInput")
    cand_d = nc.dram_tensor("cand", [B, ND * 8], f32, kind="ExternalOutput")
    araw_d = nc.dram_tensor("araw", [B, ARAW_W], bf16, kind="ExternalOutput")

    with tile.TileContext(nc) as tc:
        with tc.tile_pool(name="qp", bufs=1) as qp, \
             tc.tile_pool(name="gp", bufs=2) as gp, \
             tc.tile_pool(name="cp", bufs=8) as cp, \
             tc.tile_pool(name="sp", bufs=4) as sp, \
             tc.tile_pool(name="pp", bufs=1, space="PSUM") as pp:
            q8 = qp.tile([128, 2, B], f8, tag="q8")
            nc.sync.dma_start(out=q8[:, 0, :], in_=q_d[0])
            nc.sync.dma_start(out=q8[:, 1, :], in_=q_d[1])

            cands = [cp.tile([128, ND * 8], f32, tag="cand",
                             name=f"cand{i}") for i in range(8)]

            def consume(bc, ci, w, lhs, g8, goff, pend):
                """Emit matmuls for chunk ci (width w) and its consumer."""
                lane = "D" if ci >= NCH else _LANES[ci]
                ps = pp.tile([128, CHUNK], f32, tag=f"ps{lane}", bufs=2,
                             name=f"ps{lane}_{bc}_{ci}")
                for s in range(w // TILE):
                    nc.tensor.matmul(
                        ps[:, s * TILE:(s + 1) * TILE], lhs,
                        g8[:, :, goff + s * TILE:goff + (s + 1) * TILE],
                        start=True, stop=True, perf_mode=DR)
                if lane == "D":
                    k = ND - 1 if ci >= NCH else D_CHUNKS.index(ci)
                    nc.vector.max(cands[bc][:, k * 8:(k + 1) * 8],
                                  ps[:, :w])
                    return pend
                # Act lane: bf16 copy, buffer into a paired scratch, dump
                # one DMA per completed pair (or at a lone/tail chunk).
                ai = A_CHUNKS.index(ci)
                half = ai % 2
                if pend is None:
                    pend = (sp.tile([128, 2 * CHUNK], bf16, tag="scr",
                                    bufs=8, name=f"scr_{bc}_{ci}"), [])
                scr, slots = pend
                nc.scalar.activation(out=scr[:, half * CHUNK:half * CHUNK + w],
                                     in_=ps[:, :w], func=Copy)
                slots.append((ai, w))
                # flush on a full pair, on the lone A (ai==12), or tail
                if half == 1 or ai >= NA - 1:
                    a0, w0 = slots[0]
                    wtot = sum(wi for _, wi in slots)
                    nc.sync.dma_start(
                        out=araw_d[bc * 128:(bc + 1) * 128,
                                   a0 * CHUNK:a0 * CHUNK + wtot],
                        in_=scr[:, :wtot])
                    return None
                return pend

            tbase = 0
            for blk, ntile in enumerate(BLOCKS):
                cw = ntile * TILE
                c0 = tbase * TILE
                g8 = gp.tile([128, 2, cw], f8, tag=f"g8_{ntile}")
                nc.sync.dma_start(out=g8[:, 0, :], in_=g_d[0][:, c0:c0 + cw])
                nc.sync.dma_start(out=g8[:, 1, :], in_=g_d[1][:, c0:c0 + cw])
                for bc in range(8):
                    lhs = q8[:, :, bc * 128:(bc + 1) * 128]
                    pend = None
                    if ntile == 1:                  # 512 tail -> DVE lane
                        pend = consume(bc, NCH, TILE, lhs, g8, 0, pend)
                    else:
                        for j in range(4):          # four 1024-chunks
                            ci = (tbase * TILE) // CHUNK + j
                            pend = consume(bc, ci, CHUNK, lhs, g8,
                                           j * CHUNK, pend)
                    assert pend is None
                tbase += ntile

            for bc in range(8):
                nc.sync.dma_start(
                    out=cand_d[bc * 128:(bc + 1) * 128, :], in_=cands[bc][:])
    if not nc.is_finalized():
        nc.finalize()
    return nc


def _run_device(g_shards, q_packed):
    from concourse.bass_utils import run_bass_kernel_spmd
    if "nc" not in _CACHE:
        _CACHE["nc"] = _build_bass()
    nc = _CACHE["nc"]
    in_maps = [{"g": g_shards[c], "q": q_packed} for c in range(NCORES)]
    res = run_bass_kernel_spmd(nc, in_maps, list(range(NCORES)))
    cand = np.concatenate(
        [res.results[c]["cand"] for c in range(NCORES)], axis=1)
    araw = np.stack([res.results[c]["araw"] for c in range(NCORES)], axis=1)
    return cand, araw                     # araw: [B, NCORES, ARAW_W] bf16


def _run_emulated(g_shards, q_packed):
    import ml_dtypes
    qf = q_packed.astype(np.float32).reshape(256, B)
    cands, araws = [], []
    for c in range(NCORES):
        gf = g_shards[c].astype(np.float32).reshape(256, NPC_PAD)
        sim = qf.T @ gf                                   # [B, NPC_PAD]
        cd = np.empty((B, ND * 8), np.float32)
        for k, ci in enumerate(D_CHUNKS):
            blkv = sim[:, ci * CHUNK:(ci + 1) * CHUNK]
            cd[:, k * 8:(k + 1) * 8] = -np.sort(-blkv, axis=1)[:, :8]
        tailv = sim[:, NCH * CHUNK:NCH * CHUNK + TILE]
        cd[:, (ND - 1) * 8:ND * 8] = -np.sort(-tailv, axis=1)[:, :8]
        ar = np.concatenate(
            [sim[:, ci * CHUNK:(ci + 1) * CHUNK] for ci in A_CHUNKS], axis=1)
        cands.append(cd)
        araws.append(ar.astype(ml_dtypes.bfloat16))
    return np.concatenate(cands, axis=1), np.stack(araws, axis=1)


def kernel(test_features, train_features, train_labels):
    import ml_dtypes
    f8 = ml_dtypes.float8_e4m3

    test_features = np.asarray(test_features, dtype=np.float32)
    train_features = np.asarray(train_features, dtype=np.float32)
    train_labels = np.asarray(train_labels)

    # ---- host pre: fold normalizations into the query side ----
    tf64 = train_features.astype(np.float64)
    norm_d = np.maximum(np.sqrt(np.sum(tf64 * tf64, axis=0)), EPS)
    q64 = test_features.astype(np.float64)
    qn = np.sqrt(np.sum(q64 * q64, axis=1, keepdims=True))
    q_scaled = q64 / np.maximum(qn, EPS) / norm_d          # [B, D] f64
    # unit-normalized device queries: same per-query ranking as q_scaled
    row = np.sqrt(np.sum(q_scaled * q_scaled, axis=1, keepdims=True))
    q_unit = q_scaled / np.maximum(row, EPS)

    q_packed = np.ascontiguousarray(
        (q_unit.T * SQ).astype(f8).reshape(2, 128, B))
    gt8 = (train_features.T * SG).astype(f8)               # [D, N]
    g_shards = []
    for c in range(NCORES):
        sl = np.zeros((256, NPC_PAD), dtype=f8)
        sl[:, :NPC] = gt8[:, c * NPC:(c + 1) * NPC]
        g_shards.append(np.ascontiguousarray(sl.reshape(2, 128, NPC_PAD)))

    # ---- device: fp8 matmul + two-lane screen ----
    if os.environ.get("KNN_EMULATE"):
        cand, araw = _run_emulated(g_shards, q_packed)
    else:
        cand, araw = _run_device(g_shards, q_packed)
    cand = cand.astype(np.float64)            # [B, NCORES*ND*8]
    araw = araw.astype(np.float32)            # [B, NCORES, ARAW_W]

    # ---- host post: screen -> exact f64 rerank -> softmax scores ----
    # per-query witness threshold: 10th-largest device sim seen
    # ARAW_W = 13*1024 = 26*512: screen dumped sims at 512-col granularity
    a_chmax = araw.reshape(B, NCORES, 26, 512).max(axis=3)     # [B,C,26]
    dve_top1 = cand.reshape(B, NCORES, ND, 8)[:, :, :, 0]      # [B,C,ND]
    wit = np.concatenate(
        [cand, a_chmax.reshape(B, -1)], axis=1)
    tau = -np.partition(-wit, NB_KNN - 1, axis=1)[:, NB_KNN - 1]
    thresh = tau - MU                                          # [B]

    # span table: DVE chunks (1024 cols) + Act half-chunks (512 cols)
    spans = []          # (col0, col1) global
    sel_cols = []       # [B] bool per span
    for c in range(NCORES):
        base = c * NPC
        for k, ci in enumerate(D_CHUNKS):
            spans.append((base + ci * CHUNK,
                          base + min((ci + 1) * CHUNK, NPC)))
            sel_cols.append(dve_top1[:, c, k] >= thresh)
        spans.append((base + NCH * CHUNK, base + NPC))      # 512 tail (DVE)
        sel_cols.append(dve_top1[:, c, ND - 1] >= thresh)
        for h in range(26):
            ci = A_CHUNKS[h // 2]
            c0 = base + ci * CHUNK + (h % 2) * 512
            spans.append((c0, min(c0 + 512, base + NPC)))
            sel_cols.append(a_chmax[:, c, h] >= thresh)
    sel = np.stack(sel_cols, axis=1)                       # [B, nspans]

    reg_queries = {}
    for b in range(B):
        for r in np.nonzero(sel[b])[0]:
            reg_queries.setdefault(int(r), []).append(b)

    per_q_vals = [[] for _ in range(B)]
    per_q_cols = [[] for _ in range(B)]
    for r, qs in reg_queries.items():
        c0, c1 = spans[r]
        if c0 >= c1:
            continue
        block = tf64[c0:c1]                                # [w, D] view
        sims = q_scaled[qs] @ block.T                      # [nq, w] f64
        cols = np.arange(c0, c1)
        for i, b in enumerate(qs):
            per_q_vals[b].append(sims[i])
            per_q_cols[b].append(cols)

    labels = train_labels.astype(np.int64)
    scores = np.zeros((B, NUM_CLASSES), dtype=np.float64)
    for b in range(B):
        v = np.concatenate(per_q_vals[b])
        cidx = np.concatenate(per_q_cols[b])
        sel_i = np.argpartition(-v, NB_KNN - 1)[:NB_KNN]
        order = np.lexsort((cidx[sel_i], -v[sel_i]))
        sel_i = sel_i[order]
        topv = v[sel_i]
        w = np.exp(topv / T - np.max(topv) / T)
        w /= w.sum()
        np.add.at(scores[b], labels[cidx[sel_i]], w)
    return scores.astype(np.float32)


if __name__ == "__main__":
    rng = np.random.default_rng(0)
    tf = rng.standard_normal((B, D), dtype=np.float32)
    trf = rng.standard_normal((N, D), dtype=np.float32)
    trl = rng.integers(0, NUM_CLASSES, N).astype(np.int64)
    os.environ["KNN_EMULATE"] = "1"
    out = kernel(tf, trf, trl)
    print(out.shape, out.dtype, out.sum())


# revision 16
# speedup vs baseline: 1.9924x; 1.0515x over previous
"""KNN classification kernel for Trainium2 (8 NeuronCores).

Problem: B=1024 queries x N=200000 gallery, D=256, top-10 neighbors,
softmax-weighted one-hot class scores over 50 classes.

Math fold: reference computes gallery = l2norm(train.T, axis=1) -- i.e. each
feature dim d is normalized by ||train[:, d]|| over the FULL gallery. That
scale folds into the query side, so the device computes a pure matmul plus a
top-k screen; the host reranks screened spans exactly in f64.

Device (per core, gallery sharded along N into 8 x 25000, zero-padded to
25088 = 49 x 512), all in fp8-e4m3 with DoubleRow perf mode (K=256 folded
into one PE instruction at 0.5 cycles/row).  Sims live in PSUM and can only
leave through DVE or Act, so the screen is split so neither engine is the
bottleneck; per 128-query batch, the 24.5 1024-col chunks go to two lanes
with independent double-buffered PSUM tags (so the chains don't serialize):
  DVE lane (11 chunks):    top-8 values per chunk (InstMax)
  Act lane  (13.5 chunks): bf16 copy to SBUF, then DMA to DRAM; the host
                           screens those sims directly (bf16 error ~8 device
                           units << margin)
Host: tau_q = 10th-largest device sim among all witnesses (DVE top-8s and
  dumped bf16 sims); exactly rerank (f64) every span whose max-witness >=
  tau - mu (mu = 450 device units ~ 12 sigmas of the measured fp8 matmul
  noise); exact top-10 -> softmax -> class scores, identical math to the
  reference.
"""

import os
import numpy as np

NB_KNN = 10
T = 0.07
NUM_CLASSES = 50
EPS = 1e-12

B, N, D = 1024, 200000, 256
NCORES = 8
NPC = N // NCORES          # 25000 real cols per core
TILE = 512
NPC_PAD = 25088            # 49 * 512
CHUNK = 1024               # consumer chunk width
NCH = 24                   # full 1024-chunks per core (+ one 512 tail)
# gallery DMA blocks as (tile_start, ntiles): the 512 tail loads first (DVE
# screens it during warmup), then small lead-in blocks so Act starts early
BLOCKS = [(48, 1), (0, 4), (4, 4), (8, 8), (16, 8), (24, 8), (32, 8), (40, 8)]

SQ = 64.0                  # query fp8 pre-scale
SG = 16.0                  # gallery fp8 pre-scale
MU = 450.0                 # fp8-noise screen margin, device units

# chunk lanes per 4-chunk gallery block: A,A,D,D x5 blocks, then A,A,A,D,
# then the 512 tail -> D.  A pairs share one dump DMA.
_LANES = (["A", "A", "D", "D"] * 2 + ["A", "A", "A", "D"]
          + ["A", "A", "D", "D"] * 3)
D_CHUNKS = [i for i, l in enumerate(_LANES) if l == "D"]   # 11 chunks
A_CHUNKS = [i for i, l in enumerate(_LANES) if l == "A"]   # 13 chunks
ND = len(D_CHUNKS) + 1       # + the 512 tail chunk
NA = len(A_CHUNKS)
ARAW_W = NA * CHUNK          # 13312 bf16 sims dumped per (query, core)

_CACHE = {}


def _build_bass():
    import concourse.bacc as bacc
    import concourse.tile as tile
    from concourse import mybir

    nc = bacc.Bacc("TRN2")
    f8 = mybir.dt.float8e4
    bf16 = mybir.dt.bfloat16
    f32 = mybir.dt.float32
    DR = mybir.MatmulPerfMode.DoubleRow
    Copy = mybir.ActivationFunctionType.Copy

    g_d = nc.dram_tensor("g", [2, 128, NPC_PAD], f8, kind="ExternalInput")
    q_d = nc.dram_tensor("q", [2, 128, B], f8, kind="ExternalInput")
    cand_d = nc.dram_tensor("cand", [B, ND * 8], f32, kind="ExternalOutput")
    araw_d = nc.dram_tensor("araw", [B, ARAW_W], bf16, kind="ExternalOutput")

    with tile.TileContext(nc) as tc:
        with tc.tile_pool(name="qp", bufs=1) as qp, \
             tc.tile_pool(name="gp", bufs=2) as gp, \
             tc.tile_pool(name="cp", bufs=8) as cp, \
             tc.tile_pool(name="sp", bufs=4) as sp, \
             tc.tile_pool(name="pp", bufs=1, space="PSUM") as pp:
            q8 = qp.tile([128, 2, B], f8, tag="q8")
            nc.sync.dma_start(out=q8[:],
                              in_=q_d[:].rearrange("a p b -> p a b"))

            cands = [cp.tile([128, ND * 8], f32, tag="cand",
                             name=f"cand{i}") for i in range(8)]

            def consume(bc, ci, w, lhs, g8, goff, pend, last=False):
                """Emit matmuls for chunk ci (width w) and its consumer."""
                lane = "D" if ci >= NCH else _LANES[ci]
                ps = pp.tile([128, CHUNK], f32, tag=f"ps{lane}", bufs=2,
                             name=f"ps{lane}_{bc}_{ci}")
                for s in range(w // TILE):
                    nc.tensor.matmul(
                        ps[:, s * TILE:(s + 1) * TILE], lhs,
                        g8[:, :, goff + s * TILE:goff + (s + 1) * TILE],
                        start=True, stop=True, perf_mode=DR)
                if lane == "D":
                    k = ND - 1 if ci >= NCH else D_CHUNKS.index(ci)
                    nc.vector.max(cands[bc][:, k * 8:(k + 1) * 8],
                                  ps[:, :w])
                    return pend
                # Act lane: bf16 copy, buffer into a paired scratch, dump
                # one DMA per completed pair (or at a lone/tail chunk).
                ai = A_CHUNKS.index(ci)
                if pend is None:
                    pend = (sp.tile([128, 2 * CHUNK], bf16, tag="scr",
                                    bufs=8, name=f"scr_{bc}_{ci}"), [])
                scr, slots = pend
                half = len(slots)
                nc.scalar.activation(out=scr[:, half * CHUNK:half * CHUNK + w],
                                     in_=ps[:, :w], func=Copy)
                slots.append((ai, w))
                # flush on a full pair, or when the next chunk is not an A
                # continuation (block end / lane switch)
                nxt_a = ci + 1 < NCH and _LANES[ci + 1] == "A" and not last
                if len(slots) == 2 or not nxt_a:
                    a0, w0 = slots[0]
                    wtot = sum(wi for _, wi in slots)
                    nc.sync.dma_start(
                        out=araw_d[bc * 128:(bc + 1) * 128,
                                   a0 * CHUNK:a0 * CHUNK + wtot],
                        in_=scr[:, :wtot])
                    return None
                return pend

            for blk, (t0, ntile) in enumerate(BLOCKS):
                cw = ntile * TILE
                c0 = t0 * TILE
                g8 = gp.tile([128, 2, cw], f8, tag=f"g8_{blk}", bufs=1,
                             name=f"g8_{blk}")
                nc.vector.dma_start(out=g8[:, 0, :],
                                    in_=g_d[0][:, c0:c0 + cw])
                nc.vector.dma_start(out=g8[:, 1, :],
                                    in_=g_d[1][:, c0:c0 + cw])
                for bc in range(8):
                    lhs = q8[:, :, bc * 128:(bc + 1) * 128]
                    pend = None
                    if ntile == 1:                  # 512 tail -> DVE lane
                        pend = consume(bc, NCH, TILE, lhs, g8, 0, pend)
                    else:
                        for j in range(ntile // 2):  # 1024-chunks
                            ci = (t0 * TILE) // CHUNK + j
                            pend = consume(bc, ci, CHUNK, lhs, g8,
                                           j * CHUNK, pend,
                                           last=(j == ntile // 2 - 1))
                            if ci == NCH - 1:        # bc's last DVE chunk
                                nc.sync.dma_start(
                                    out=cand_d[bc * 128:(bc + 1) * 128, :],
                                    in_=cands[bc][:])
                    assert pend is None
    if not nc.is_finalized():
        nc.finalize()
    return nc


def _run_device(g_shards, q_packed):
    from concourse.bass_utils import run_bass_kernel_spmd
    if "nc" not in _CACHE:
        _CACHE["nc"] = _build_bass()
    nc = _CACHE["nc"]
    in_maps = [{"g": g_shards[c], "q": q_packed} for c in range(NCORES)]
    res = run_bass_kernel_spmd(nc, in_maps, list(range(NCORES)))
    cand = np.concatenate(
        [res.results[c]["cand"] for c in range(NCORES)], axis=1)
    araw = np.stack([res.results[c]["araw"] for c in range(NCORES)], axis=1)
    return cand, araw                     # araw: [B, NCORES, ARAW_W] bf16


def _run_emulated(g_shards, q_packed):
    import ml_dtypes
    qf = q_packed.astype(np.float32).reshape(256, B)
    cands, araws = [], []
    for c in range(NCORES):
        gf = g_shards[c].astype(np.float32).reshape(256, NPC_PAD)
        sim = qf.T @ gf                                   # [B, NPC_PAD]
        cd = np.empty((B, ND * 8), np.float32)
        for k, ci in enumerate(D_CHUNKS):
            blkv = sim[:, ci * CHUNK:(ci + 1) * CHUNK]
            cd[:, k * 8:(k + 1) * 8] = -np.sort(-blkv, axis=1)[:, :8]
        tailv = sim[:, NCH * CHUNK:NCH * CHUNK + TILE]
        cd[:, (ND - 1) * 8:ND * 8] = -np.sort(-tailv, axis=1)[:, :8]
        ar = np.concatenate(
            [sim[:, ci * CHUNK:(ci + 1) * CHUNK] for ci in A_CHUNKS], axis=1)
        cands.append(cd)
        araws.append(ar.astype(ml_dtypes.bfloat16))
    return np.concatenate(cands, axis=1), np.stack(araws, axis=1)


def kernel(test_features, train_features, train_labels):
    import ml_dtypes
    f8 = ml_dtypes.float8_e4m3

    test_features = np.asarray(test_features, dtype=np.float32)
    train_features = np.asarray(train_features, dtype=np.float32)
    train_labels = np.asarray(train_labels)

    # ---- host pre: fold normalizations into the query side ----
    tf64 = train_features.astype(np.float64)
    norm_d = np.maximum(np.sqrt(np.sum(tf64 * tf64, axis=0)), EPS)
    q64 = test_features.astype(np.float64)
    qn = np.sqrt(np.sum(q64 * q64, axis=1, keepdims=True))
    q_scaled = q64 / np.maximum(qn, EPS) / norm_d          # [B, D] f64
    # unit-normalized device queries: same per-query ranking as q_scaled
    row = np.sqrt(np.sum(q_scaled * q_scaled, axis=1, keepdims=True))
    q_unit = q_scaled / np.maximum(row, EPS)

    q_packed = np.ascontiguousarray(
        (q_unit.T * SQ).astype(f8).reshape(2, 128, B))
    gt8 = (train_features.T * SG).astype(f8)               # [D, N]
    g_shards = []
    for c in range(NCORES):
        sl = np.zeros((256, NPC_PAD), dtype=f8)
        sl[:, :NPC] = gt8[:, c * NPC:(c + 1) * NPC]
        g_shards.append(np.ascontiguousarray(sl.reshape(2, 128, NPC_PAD)))

    # ---- device: fp8 matmul + two-lane screen ----
    if os.environ.get("KNN_EMULATE"):
        cand, araw = _run_emulated(g_shards, q_packed)
    else:
        cand, araw = _run_device(g_shards, q_packed)
    cand = cand.astype(np.float64)            # [B, NCORES*ND*8]
    araw = araw.astype(np.float32)            # [B, NCORES, ARAW_W]

    # ---- host post: screen -> exact f64 rerank -> softmax scores ----
    # per-query witness threshold: 10th-largest device sim seen
    # ARAW_W = 13*1024 = 26*512: screen dumped sims at 512-col granularity
    a_chmax = araw.reshape(B, NCORES, 26, 512).max(axis=3)     # [B,C,26]
    dve_top1 = cand.reshape(B, NCORES, ND, 8)[:, :, :, 0]      # [B,C,ND]
    wit = np.concatenate(
        [cand, a_chmax.reshape(B, -1)], axis=1)
    tau = -np.partition(-wit, NB_KNN - 1, axis=1)[:, NB_KNN - 1]
    thresh = tau - MU                                          # [B]

    # span table: DVE chunks (1024 cols) + Act half-chunks (512 cols)
    spans = []          # (col0, col1) global
    sel_cols = []       # [B] bool per span
    for c in range(NCORES):
        base = c * NPC
        for k, ci in enumerate(D_CHUNKS):
            spans.append((base + ci * CHUNK,
                          base + min((ci + 1) * CHUNK, NPC)))
            sel_cols.append(dve_top1[:, c, k] >= thresh)
        spans.append((base + NCH * CHUNK, base + NPC))      # 512 tail (DVE)
        sel_cols.append(dve_top1[:, c, ND - 1] >= thresh)
        for h in range(26):
            ci = A_CHUNKS[h // 2]
            c0 = base + ci * CHUNK + (h % 2) * 512
            spans.append((c0, min(c0 + 512, base + NPC)))
            sel_cols.append(a_chmax[:, c, h] >= thresh)
    sel = np.stack(sel_cols, axis=1)                       # [B, nspans]

    reg_queries = {}
    for b in range(B):
        for r in np.nonzero(sel[b])[0]:
            reg_queries.setdefault(int(r), []).append(b)

    per_q_vals = [[] for _ in range(B)]
    per_q_cols = [[] for _ in range(B)]
    for r, qs in reg_queries.items():
        c0, c1 = spans[r]
        if c0 >= c1:
            continue
        block = tf64[c0:c1]                                # [w, D] view
        sims = q_scaled[qs] @ block.T                      # [nq, w] f64
        cols = np.arange(c0, c1)
        for i, b in enumerate(qs):
            per_q_vals[b].append(sims[i])
            per_q_cols[b].append(cols)

    labels = train_labels.astype(np.int64)
    scores = np.zeros((B, NUM_CLASSES), dtype=np.float64)
    for b in range(B):
        v = np.concatenate(per_q_vals[b])
        cidx = np.concatenate(per_q_cols[b])
        sel_i = np.argpartition(-v, NB_KNN - 1)[:NB_KNN]
        order = np.lexsort((cidx[sel_i], -v[sel_i]))
        sel_i = sel_i[order]
        topv = v[sel_i]
        w = np.exp(topv / T - np.max(topv) / T)
        w /= w.sum()
        np.add.at(scores[b], labels[cidx[sel_i]], w)
    return scores.astype(np.float32)


if __name__ == "__main__":
    rng = np.random.default_rng(0)
    tf = rng.standard_normal((B, D), dtype=np.float32)
    trf = rng.standard_normal((N, D), dtype=np.float32)
    trl = rng.integers(0, NUM_CLASSES, N).astype(np.int64)
    os.environ["KNN_EMULATE"] = "1"
    out = kernel(tf, trf, trl)
    print(out.shape, out.dtype, out.sum())


# revision 24
# speedup vs baseline: 2.0151x; 1.0114x over previous
"""KNN classification kernel for Trainium2 (8 NeuronCores).

Problem: B=1024 queries x N=200000 gallery, D=256, top-10 neighbors,
softmax-weighted one-hot class scores over 50 classes.

Math fold: reference computes gallery = l2norm(train.T, axis=1) -- i.e. each
feature dim d is normalized by ||train[:, d]|| over the FULL gallery. That
scale folds into the query side, so the device computes a pure matmul plus a
top-k screen; the host reranks screened spans exactly in f64.

Device (per core, gallery sharded along N into 8 x 25000, zero-padded to
25088 = 49 x 512), all in fp8-e4m3 with DoubleRow perf mode (K=256 folded
into one PE instruction at 0.5 cycles/row).  Sims live in PSUM and can only
leave through DVE or Act, so the screen is split so neither engine is the
bottleneck; per 128-query batch, the 24.5 1024-col chunks go to two lanes
with independent double-buffered PSUM tags (so the chains don't serialize):
  DVE lane (11 chunks):    top-8 values per chunk (InstMax)
  Act lane  (13.5 chunks): bf16 copy to SBUF, then DMA to DRAM; the host
                           screens those sims directly (bf16 error ~8 device
                           units << margin)
Host: tau_q = 10th-largest device sim among all witnesses (DVE top-8s and
  dumped bf16 sims); exactly rerank (f64) every span whose max-witness >=
  tau - mu (mu = 450 device units ~ 12 sigmas of the measured fp8 matmul
  noise); exact top-10 -> softmax -> class scores, identical math to the
  reference.
"""

import os
import numpy as np

NB_KNN = 10
T = 0.07
NUM_CLASSES = 50
EPS = 1e-12

B, N, D = 1024, 200000, 256
NCORES = 8
NPC = N // NCORES          # 25000 real cols per core
TILE = 512
NPC_PAD = 25088            # 49 * 512
CHUNK = 1024               # consumer chunk width
NCH = 24                   # full 1024-chunks per core (+ one 512 tail)
# gallery DMA blocks as (tile_start, ntiles): the 512 tail loads first (DVE
# screens it during warmup), then small lead-in blocks so Act starts early
BLOCKS = [(48, 1), (0, 4), (4, 4), (8, 8), (16, 8), (24, 8), (32, 8), (40, 8)]

SQ = 64.0                  # query fp8 pre-scale
SG = 16.0                  # gallery fp8 pre-scale
MU = 450.0                 # fp8-noise screen margin, device units

# chunk lanes per 4-chunk gallery block: A,A,D,D x5 blocks, then A,A,A,D,
# then the 512 tail -> D.  A pairs share one dump DMA.
_LANES = (["A", "D", "A", "D"] * 2 + ["A", "A", "A", "D"]
          + ["A", "D", "A", "D"] * 3)
D_CHUNKS = [i for i, l in enumerate(_LANES) if l == "D"]   # 11 chunks
A_CHUNKS = [i for i, l in enumerate(_LANES) if l == "A"]   # 13 chunks
ND = len(D_CHUNKS) + 1       # + the 512 tail chunk
NA = len(A_CHUNKS)
ARAW_W = NA * CHUNK          # 13312 bf16 sims dumped per (query, core)

_CACHE = {}


def _build_bass():
    import concourse.bacc as bacc
    import concourse.tile as tile
    from concourse import mybir

    nc = bacc.Bacc("TRN2")
    f8 = mybir.dt.float8e4
    bf16 = mybir.dt.bfloat16
    f32 = mybir.dt.float32
    DR = mybir.MatmulPerfMode.DoubleRow
    Copy = mybir.ActivationFunctionType.Copy

    g_d = nc.dram_tensor("g", [2, 128, NPC_PAD], f8, kind="ExternalInput")
    q_d = nc.dram_tensor("q", [2, 128, B], f8, kind="ExternalInput")
    cand_d = nc.dram_tensor("cand", [B, ND * 8], f32, kind="ExternalOutput")
    araw_d = nc.dram_tensor("araw", [B, ARAW_W], bf16, kind="ExternalOutput")

    with tile.TileContext(nc) as tc:
        with tc.tile_pool(name="qp", bufs=1) as qp, \
             tc.tile_pool(name="gp", bufs=2) as gp, \
             tc.tile_pool(name="cp", bufs=8) as cp, \
             tc.tile_pool(name="sp", bufs=4) as sp, \
             tc.tile_pool(name="pp", bufs=1, space="PSUM") as pp:
            q8 = qp.tile([128, 2, B], f8, tag="q8")
            nc.sync.dma_start(out=q8[:],
                              in_=q_d[:].rearrange("a p b -> p a b"))

            cands = [cp.tile([128, ND * 8], f32, tag="cand",
                             name=f"cand{i}") for i in range(8)]

            def consume(bc, ci, w, lhs, g8, goff, pend, last=False,
                        scan_w=None):
                """Emit matmuls for chunk ci (width w) and its consumer."""
                lane = "D" if ci >= NCH else _LANES[ci]
                ps = pp.tile([128, CHUNK], f32, tag=f"ps{lane}", bufs=2,
                             name=f"ps{lane}_{bc}_{ci}")
                for s in range(w // TILE):
                    nc.tensor.matmul(
                        ps[:, s * TILE:(s + 1) * TILE], lhs,
                        g8[:, :, goff + s * TILE:goff + (s + 1) * TILE],
                        start=True, stop=True, perf_mode=DR)
                if lane == "D":
                    k = ND - 1 if ci >= NCH else D_CHUNKS.index(ci)
                    nc.vector.max(cands[bc][:, k * 8:(k + 1) * 8],
                                  ps[:, :(scan_w or w)])
                    return pend
                # Act lane: bf16 copy, buffer into a paired scratch, dump
                # one DMA per completed pair (or at a lone/tail chunk).
                ai = A_CHUNKS.index(ci)
                if pend is None:
                    pend = (sp.tile([128, 2 * CHUNK], bf16, tag="scr",
                                    bufs=8, name=f"scr_{bc}_{ci}"), [])
                scr, slots = pend
                half = len(slots)
                nc.scalar.activation(out=scr[:, half * CHUNK:half * CHUNK + w],
                                     in_=ps[:, :w], func=Copy)
                slots.append((ai, w))
                # flush on a full pair, or when the next chunk is not an A
                # continuation (block end / lane switch)
                nxt_a = ci + 1 < NCH and _LANES[ci + 1] == "A" and not last
                if len(slots) == 2 or not nxt_a:
                    a0, w0 = slots[0]
                    wtot = sum(wi for _, wi in slots)
                    nc.sync.dma_start(
                        out=araw_d[bc * 128:(bc + 1) * 128,
                                   a0 * CHUNK:a0 * CHUNK + wtot],
                        in_=scr[:, :wtot])
                    return None
                return pend

            for blk, (t0, ntile) in enumerate(BLOCKS):
                cw = ntile * TILE
                c0 = t0 * TILE
                g8 = gp.tile([128, 2, cw], f8, tag=f"g8_{blk}", bufs=1,
                             name=f"g8_{blk}")
                nc.sync.dma_start(
                    out=g8[:],
                    in_=g_d[:, :, c0:c0 + cw].rearrange("a p b -> p a b"))
                for bc in range(8):
                    lhs = q8[:, :, bc * 128:(bc + 1) * 128]
                    pend = None
                    if ntile == 1:                  # 512 tail -> DVE lane
                        pend = consume(bc, NCH, TILE, lhs, g8, 0, pend,
                                       scan_w=NPC - 48 * TILE)
                    else:
                        for j in range(ntile // 2):  # 1024-chunks
                            ci = (t0 * TILE) // CHUNK + j
                            pend = consume(bc, ci, CHUNK, lhs, g8,
                                           j * CHUNK, pend,
                                           last=(j == ntile // 2 - 1))
                            if ci == 19:             # most DVE slots done
                                nc.sync.dma_start(
                                    out=cand_d[bc * 128:(bc + 1) * 128, :72],
                                    in_=cands[bc][:, :72])
                            if ci == NCH - 1:        # bc's last DVE chunk
                                nc.sync.dma_start(
                                    out=cand_d[bc * 128:(bc + 1) * 128, 72:],
                                    in_=cands[bc][:, 72:])
                    assert pend is None
    if not nc.is_finalized():
        nc.finalize()
    return nc


def _run_device(g_shards, q_packed):
    from concourse.bass_utils import run_bass_kernel_spmd
    if "nc" not in _CACHE:
        _CACHE["nc"] = _build_bass()
    nc = _CACHE["nc"]
    in_maps = [{"g": g_shards[c], "q": q_packed} for c in range(NCORES)]
    res = run_bass_kernel_spmd(nc, in_maps, list(range(NCORES)))
    cand = np.concatenate(
        [res.results[c]["cand"] for c in range(NCORES)], axis=1)
    araw = np.stack([res.results[c]["araw"] for c in range(NCORES)], axis=1)
    return cand, araw                     # araw: [B, NCORES, ARAW_W] bf16


def _run_emulated(g_shards, q_packed):
    import ml_dtypes
    qf = q_packed.astype(np.float32).reshape(256, B)
    cands, araws = [], []
    for c in range(NCORES):
        gf = g_shards[c].astype(np.float32).reshape(256, NPC_PAD)
        sim = qf.T @ gf                                   # [B, NPC_PAD]
        cd = np.empty((B, ND * 8), np.float32)
        for k, ci in enumerate(D_CHUNKS):
            blkv = sim[:, ci * CHUNK:(ci + 1) * CHUNK]
            cd[:, k * 8:(k + 1) * 8] = -np.sort(-blkv, axis=1)[:, :8]
        tailv = sim[:, NCH * CHUNK:NCH * CHUNK + TILE]
        cd[:, (ND - 1) * 8:ND * 8] = -np.sort(-tailv, axis=1)[:, :8]
        ar = np.concatenate(
            [sim[:, ci * CHUNK:(ci + 1) * CHUNK] for ci in A_CHUNKS], axis=1)
        cands.append(cd)
        araws.append(ar.astype(ml_dtypes.bfloat16))
    return np.concatenate(cands, axis=1), np.stack(araws, axis=1)


def kernel(test_features, train_features, train_labels):
    import ml_dtypes
    f8 = ml_dtypes.float8_e4m3

    test_features = np.asarray(test_features, dtype=np.float32)
    train_features = np.asarray(train_features, dtype=np.float32)
    train_labels = np.asarray(train_labels)

    # ---- host pre: fold normalizations into the query side ----
    tf64 = train_features.astype(np.float64)
    norm_d = np.maximum(np.sqrt(np.sum(tf64 * tf64, axis=0)), EPS)
    q64 = test_features.astype(np.float64)
    qn = np.sqrt(np.sum(q64 * q64, axis=1, keepdims=True))
    q_scaled = q64 / np.maximum(qn, EPS) / norm_d          # [B, D] f64
    # unit-normalized device queries: same per-query ranking as q_scaled
    row = np.sqrt(np.sum(q_scaled * q_scaled, axis=1, keepdims=True))
    q_unit = q_scaled / np.maximum(row, EPS)

    q_packed = np.ascontiguousarray(
        (q_unit.T * SQ).astype(f8).reshape(2, 128, B))
    gt8 = (train_features.T * SG).astype(f8)               # [D, N]
    g_shards = []
    for c in range(NCORES):
        sl = np.zeros((256, NPC_PAD), dtype=f8)
        sl[:, :NPC] = gt8[:, c * NPC:(c + 1) * NPC]
        g_shards.append(np.ascontiguousarray(sl.reshape(2, 128, NPC_PAD)))

    # ---- device: fp8 matmul + two-lane screen ----
    if os.environ.get("KNN_EMULATE"):
        cand, araw = _run_emulated(g_shards, q_packed)
    else:
        cand, araw = _run_device(g_shards, q_packed)
    cand = cand.astype(np.float64)            # [B, NCORES*ND*8]
    araw = araw.astype(np.float32)            # [B, NCORES, ARAW_W]

    # ---- host post: screen -> exact f64 rerank -> softmax scores ----
    # per-query witness threshold: 10th-largest device sim seen
    # ARAW_W = 13*1024 = 26*512: screen dumped sims at 512-col granularity
    a_chmax = araw.reshape(B, NCORES, 26, 512).max(axis=3)     # [B,C,26]
    dve_top1 = cand.reshape(B, NCORES, ND, 8)[:, :, :, 0]      # [B,C,ND]
    wit = np.concatenate(
        [cand, a_chmax.reshape(B, -1)], axis=1)
    tau = -np.partition(-wit, NB_KNN - 1, axis=1)[:, NB_KNN - 1]
    thresh = tau - MU                                          # [B]

    # span table: DVE chunks (1024 cols) + Act half-chunks (512 cols)
    spans = []          # (col0, col1) global
    sel_cols = []       # [B] bool per span
    for c in range(NCORES):
        base = c * NPC
        for k, ci in enumerate(D_CHUNKS):
            spans.append((base + ci * CHUNK,
                          base + min((ci + 1) * CHUNK, NPC)))
            sel_cols.append(dve_top1[:, c, k] >= thresh)
        spans.append((base + NCH * CHUNK, base + NPC))      # 512 tail (DVE)
        sel_cols.append(dve_top1[:, c, ND - 1] >= thresh)
        for h in range(26):
            ci = A_CHUNKS[h // 2]
            c0 = base + ci * CHUNK + (h % 2) * 512
            spans.append((c0, min(c0 + 512, base + NPC)))
            sel_cols.append(a_chmax[:, c, h] >= thresh)
    sel = np.stack(sel_cols, axis=1)                       # [B, nspans]

    reg_queries = {}
    for b in range(B):
        for r in np.nonzero(sel[b])[0]:
            reg_queries.setdefault(int(r), []).append(b)

    per_q_vals = [[] for _ in range(B)]
    per_q_cols = [[] for _ in range(B)]
    for r, qs in reg_queries.items():
        c0, c1 = spans[r]
        if c0 >= c1:
            continue
        block = tf64[c0:c1]                                # [w, D] view
        sims = q_scaled[qs] @ block.T                      # [nq, w] f64
        cols = np.arange(c0, c1)
        for i, b in enumerate(qs):
            per_q_vals[b].append(sims[i])
            per_q_cols[b].append(cols)

    labels = train_labels.astype(np.int64)
    scores = np.zeros((B, NUM_CLASSES), dtype=np.float64)
    for b in range(B):
        v = np.concatenate(per_q_vals[b])
        cidx = np.concatenate(per_q_cols[b])
        sel_i = np.argpartition(-v, NB_KNN - 1)[:NB_KNN]
        order = np.lexsort((cidx[sel_i], -v[sel_i]))
        sel_i = sel_i[order]
        topv = v[sel_i]
        w = np.exp(topv / T - np.max(topv) / T)
        w /= w.sum()
        np.add.at(scores[b], labels[cidx[sel_i]], w)
    return scores.astype(np.float32)


if __name__ == "__main__":
    rng = np.random.default_rng(0)
    tf = rng.standard_normal((B, D), dtype=np.float32)
    trf = rng.standard_normal((N, D), dtype=np.float32)
    trl = rng.integers(0, NUM_CLASSES, N).astype(np.int64)
    os.environ["KNN_EMULATE"] = "1"
    out = kernel(tf, trf, trl)
    print(out.shape, out.dtype, out.sum())
